# revision 10
# baseline (speedup 1.0000x reference)
"""Trainium2 Bass kernel for nn_ButterflyFactorNewMlp.

Computes: attn = einsum('ds,td->st', w1, w2) * sparse_mask
          out  = gelu(einsum('bds,st->bdt', x, attn) + b2)   (exact erf gelu)

Key structural fact (hardcoded): mask[s,t] != 0  iff  s//81 == t//81 and
(s%27)//3 == (t%27)//3.  Writing s = 81A + 27B + 3C + c, that is "same
(A,C)".  Under the permutation u = 81A + 9C + 3B + c (sort by (A,C)), the
masked attn becomes BLOCK-DIAGONAL with 81 dense 9x9 blocks.  Grouping 14
blocks per 126-wide chunk (5 chunks of 126 + 1 of 99), the main einsum
decomposes into 6 independent [cw x cw] matmuls per token tile -- a ~4x cut
in PE work versus exploiting only the 9x81x81 block structure, with zero
wasted stream columns.

Layout: the host pre-permutes and pre-TRANSPOSES x into xT [768, 6144] per
core (chunk j at rows 128j:128j+cw, zero pad between) so the contraction
dim (s) is already on partitions -- no on-device transposes -- and every
DMA moves full 128-partition tiles, which the runtime splits evenly over
all 16 SDMA engines (126-partition transfers leave engines idle).  The
matmul streams xT token tiles against a stationary attn chunk, producing
out in [t, token] layout; gelu+bias runs on ScalarE straight out of PSUM
(bias is per-partition here, so it fuses into the activation for free) over
a 2-bank [cw, 1024] window to amortize the ~370ns fixed access latency.
The fp16 store goes back t-major; the host transposes/unpermutes.

Pipelining: x loads issue on the SP HWDGE ring, output stores + w2 on the
ACT ring -- out stores then queue right behind their activations with no
cross-tile head-of-line blocking of the x stream (this alone was worth
~20us).  The weight stream owns most DMA queues
early; x prefetch is capped at bufs=2 so deferred x bytes ride stage 2's
bus slack (stage 2 is ScalarE-paced, ~85% bus duty).  A/B
chunk-group weight splitting was tried and REGRESSED (121us): 12->24
weight transfers starve the later group behind the stage-2 stream, and
the in-order PE then stalls every queued tile behind stage-1B.  A dummy
activation right after the (early, tiny) b2 load pulls the ~1.3us gelu
ACT-table load off the critical path.

Sharding: data-parallel on batch (8 batches = 6144 tokens per core); the
small attn computation is replicated on every core (fp16 weights, ~8.6MB
DMA) -- measured previously, this beats d-sharding + AllReduce (any
on-device collective drags in ~100us of ncfw startup + launch-skew barrier
+ latency-bound AllReduce).

Precision: x and weights in fp16 (fp8 measured: 3.1e-2 max rel err, over
the 2e-2 gate -- rejected), fp32 PSUM accumulation, exact-erf gelu LUT on
ScalarE, fp16 stores.  End-to-end max rel err ~7.5e-4.
"""

import sys

if "/opt/trn_rl_repo" not in sys.path:
    sys.path.insert(0, "/opt/trn_rl_repo")

import numpy as np

import concourse.bacc as bacc
import concourse.mybir as mybir
import concourse.tile as tile
from concourse.bass import ds
from concourse.bass_utils import run_bass_kernel_spmd

F32 = mybir.dt.float32
F16 = mybir.dt.float16
GELU = mybir.ActivationFunctionType.Gelu

N_CORES = 8
B, D, S = 64, 768, 729          # batch, channels, features (729 = in = out)
H = 2916                        # hidden dim of the weight contraction
HP = 2944                       # hidden padded to 23*128
N_KD = HP // 128                # 23 contraction chunks for the attn matmuls
KD_BATCH = 4                    # kd chunks per weight DMA
M_PER_CORE = (B // N_CORES) * D  # 6144 tokens per core
SPAD = 768                      # padded feature rows: 6 chunks x 128
# token tiles: 1024-token steady state, 512 tails to shrink the drain
T_TILES = [(0, 1024), (1024, 1024), (2048, 1024), (3072, 1024), (4096, 1024),
           (5120, 512), (5632, 512)]
T_SUB = 512                     # tokens per matmul (PSUM bank = 512 f32)
CW = [126, 126, 126, 126, 126, 99]  # chunk widths (14*9 x5, 11*9)
NCH = 6

_COMPILED = None
LAST = None  # BassKernelResults of the most recent kernel() call (for test.py)


def _perm():
    u = np.arange(S)
    g, r = u // 9, u % 9
    return 81 * (g // 9) + 27 * (r // 3) + 3 * (g % 9) + (r % 3)


def _build():
    nc = bacc.Bacc("TRN2", target_bir_lowering=False, debug=False)

    xT_d = nc.dram_tensor("xT", [SPAD, M_PER_CORE], F16, kind="ExternalInput")
    w1p_d = nc.dram_tensor("w1p", [HP, S], F16, kind="ExternalInput")
    w2p_d = nc.dram_tensor("w2p", [HP, S], F16, kind="ExternalInput")
    mp_d = nc.dram_tensor("maskp", [126, NCH, 126], F16, kind="ExternalInput")
    b2p_d = nc.dram_tensor("b2p", [126, NCH], F32, kind="ExternalInput")
    outT_d = nc.dram_tensor("outT", [SPAD, M_PER_CORE], F16, kind="ExternalOutput")

    with tile.TileContext(nc) as tc:
        with (
            tc.tile_pool(name="const", bufs=1) as cpool,
            tc.tile_pool(name="xin", bufs=2) as xpool,
            tc.tile_pool(name="oout", bufs=3) as opool,
        ):
            # ---------------- stage 1: replicated attn ----------------
            w1_sb = cpool.tile([128, N_KD, S], F16)
            w2_sb = cpool.tile([128, N_KD, S], F16)
            for kb in range((N_KD + KD_BATCH - 1) // KD_BATCH):
                k0 = kb * KD_BATCH
                kn = min(KD_BATCH, N_KD - k0)
                nc.sync.dma_start(
                    w1_sb[:, ds(k0, kn), :],
                    w1p_d[ds(k0 * 128, kn * 128), :].rearrange(
                        "(c p) f -> p c f", p=128
                    ),
                )
                nc.scalar.dma_start(
                    w2_sb[:, ds(k0, kn), :],
                    w2p_d[ds(k0 * 128, kn * 128), :].rearrange(
                        "(c p) f -> p c f", p=128
                    ),
                )

            mp_sb = cpool.tile([126, NCH, 126], F16)
            nc.scalar.dma_start(mp_sb[:], mp_d[:])
            b2_sb = cpool.tile([126, NCH], F32)
            nc.scalar.dma_start(b2_sb[:], b2p_d[:])
            # dummy activation AFTER the weight dma issues (a wait at the
            # scalar sequencer head here no longer delays the w2 stream):
            # pulls the ~1.3us gelu ACT-table load off the critical path
            warm_sb = cpool.tile([1, NCH], F16)
            nc.scalar.activation(warm_sb[:], b2_sb[0:1, :], GELU)

            attn_sb = cpool.tile([126, NCH, 126], F16)

            # kd-outer so the 6 chunks' accumulations pipeline with the
            # incoming weight DMA stream (one small PSUM region per chunk)
            with tc.tile_pool(name="apsum", bufs=6, space="PSUM") as apsum:
                psa = [
                    apsum.tile([CW[j], CW[j]], F32, tag="aps", name=f"aps{j}")
                    for j in range(NCH)
                ]
                for kd in range(N_KD):
                    for j in range(NCH):
                        w = CW[j]
                        nc.tensor.matmul(
                            psa[j][:, :],
                            w1_sb[:, kd, ds(126 * j, w)],
                            w2_sb[:, kd, ds(126 * j, w)],
                            start=(kd == 0),
                            stop=(kd == N_KD - 1),
                        )
                for j in range(NCH):
                    w = CW[j]
                    nc.vector.tensor_tensor(
                        attn_sb[0:w, j, 0:w],
                        psa[j][:, :],
                        mp_sb[0:w, j, 0:w],
                        mybir.AluOpType.mult,
                    )

            # ---------------- stage 2: block-diag main matmul ----------
            with tc.tile_pool(name="tpsum", bufs=3, space="PSUM") as tpsum:
                for t0, tn in T_TILES:
                    nh = tn // T_SUB
                    xt = xpool.tile([128, NCH, tn], F16, tag="xt")
                    nc.sync.dma_start(
                        xt[:],
                        xT_d[:, ds(t0, tn)].rearrange("(c p) f -> p c f", p=128),
                    )
                    o_sb = opool.tile([128, NCH, tn], F16, tag="o")
                    for j in range(NCH):
                        w = CW[j]
                        pst = tpsum.tile([126, 2, T_SUB], F32, tag="tps", name="tps")
                        for h in range(nh):
                            nc.tensor.matmul(
                                pst[0:w, h, :],
                                attn_sb[0:w, j, 0:w],
                                xt[0:w, j, ds(h * T_SUB, T_SUB)],
                                start=True,
                                stop=True,
                            )
                        nc.scalar.activation(
                            o_sb[0:w, j, :],
                            pst[0:w, 0:nh, :],
                            GELU,
                            bias=b2_sb[0:w, ds(j, 1)],
                            scale=1.0,
                        )
                    nc.scalar.dma_start(
                        outT_d[:, ds(t0, tn)].rearrange("(c p) f -> p c f", p=128),
                        o_sb[:],
                    )

    nc.compile()
    return nc


def _host_prep(w1, w2, b2, perm):
    """Build the permuted fp16 weight / mask-window / bias tables."""
    w1p = np.zeros((HP, S), np.float16)
    w1p[:H] = w1[:, perm]
    w2p = np.zeros((HP, S), np.float16)
    w2p[:H] = w2.T[:, perm]
    maskp = np.zeros((126, NCH, 126), np.float16)
    for j in range(NCH):
        w = CW[j]
        blk = np.kron(np.eye(w // 9, dtype=np.float16), np.ones((9, 9), np.float16))
        maskp[0:w, j, 0:w] = blk
    b2p = np.zeros((126, NCH), np.float32)
    for j in range(NCH):
        w = CW[j]
        b2p[0:w, j] = b2[perm[126 * j : 126 * j + w]]
    return w1p, w2p, maskp, b2p


def kernel(x, w1, w2, b2, sparse_mask):
    global _COMPILED, LAST
    if _COMPILED is None:
        _COMPILED = _build()
    nc = _COMPILED

    x = np.asarray(x, dtype=np.float32)
    w1 = np.asarray(w1, dtype=np.float32)
    w2 = np.asarray(w2, dtype=np.float32)
    b2 = np.asarray(b2, dtype=np.float32)

    perm = _perm()
    w1p, w2p, maskp, b2p = _host_prep(w1, w2, b2, perm)

    xh = x.reshape(B * D, S).astype(np.float16)
    xTp = xh.T[perm]  # [729, B*D] fp16, permuted rows
    xT = np.zeros((SPAD, B * D), np.float16)
    for j in range(NCH):
        w = CW[j]
        xT[128 * j : 128 * j + w] = xTp[126 * j : 126 * j + w]

    in_maps = []
    for c in range(N_CORES):
        in_maps.append(
            {
                "xT": np.ascontiguousarray(
                    xT[:, c * M_PER_CORE : (c + 1) * M_PER_CORE]
                ),
                "w1p": w1p,
                "w2p": w2p,
                "maskp": maskp,
                "b2p": b2p,
            }
        )

    LAST = run_bass_kernel_spmd(nc, in_maps, list(range(N_CORES)))
    outT = np.concatenate(
        [LAST.results[c]["outT"] for c in range(N_CORES)], axis=1
    )  # [768, B*D] fp16
    out = np.empty((B * D, S), np.float32)
    for j in range(NCH):
        w = CW[j]
        out[:, perm[126 * j : 126 * j + w]] = outT[128 * j : 128 * j + w].T
    return out.reshape(B, D, S)
